# revision 25
# baseline (speedup 1.0000x reference)
"""Trainium2 Bass kernel for nn_MemorizedAttention.

Computes, per (batch, head):
    Q = q @ Wq ; K = [k @ Wk ; memory_k] ; V = [v @ Wv ; memory_v]
    out = softmax(Q K^T / sqrt(768)) V          (biases are all zero)

Sharding: 24 (batch*head) units data-parallel over 8 cores (3 heads/core).
Weights / memory tokens replicated.

Steady-state design (per core, per head, 4 q-blocks of 512 queries):
  - Memory keys are folded out algebraically: their scores satisfy
    |s*scale| <= ~0.1, so exp(x) ~= 1+x (rel err ~3e-4) and the whole
    memory contribution -- including its part of the softmax denominator
    -- collapses into one precomputed 65x65 matrix
    M2 = [scale*mk ; 1]^T [mv | 1] applied to [Q ; 1] per q-block: one
    512-cycle matmul replacing ~2.3 chunks of QK+exp+PV work. Exactly 16
    full 128-key text chunks remain (2048 = 16*128, no partial chunk).
  - QK on PE in fp8e4 DoubleRow, operands zero-padded to 128 partitions
    (64-partition matmuls run at half stream rate on this HW): stationary
    K^T chunk planes (K8, K-K8 requantized -- one-sided error feedback),
    moving Q^T planes (Q8, Q8); one matmul per chunk -> PSUM [128k, 512q]
    transposed scores in 2-chunk slot tiles over three rotating slots.
  - Softmax exp on ACT (exp -> fp16, scale fused); no max subtraction
    (|scores*scale| < ~3). PV accumulates outT[65, 512] += V_c^T P_c in
    PSUM over all 16 chunks then the M2 term closes the group; V column
    64 is ones so outT row 64 is the denominator.
  - Normalize: outT -> SBUF fp16 (DVE), 4 PE transposes bracketed into
    one single-bank accumulation group (fp16 identity), DVE reciprocal +
    per-partition scale, DMA out.
  - Flat software pipeline over (head, qblock, group) items with PV
    lagging QK by 2 groups so the batched exp latency stays off PE's
    critical path.

HW notes (measured on trn2 via paired-loop timing; CoreSim underestimates):
  - Matmuls whose operands span only 64 partitions run their moving
    stream at HALF rate (~2 cyc/col fp16): QK with contraction 64 cost
    ~460ns/chunk. Zero-padding Q^T/K^T to 128 partitions (rows 64+ = 0;
    row 64 of Q^T doubles as the ones-row for M2) restores full rate
    (~235ns/chunk). This single change is worth ~40us/iteration.
  - fp8e4 DoubleRow QK (0.5 cyc/col in the cost model) is NOT faster on
    this toolchain: DR ldweights is not hidden and mixing DR with fp16
    matmuls in one PE stream costs ~1us per mode switch.
  - PSUM zero-regions are per-bank: matmuls of one start/stop bracket
    must stay within one 2KB bank (PV chains and the transpose bracket
    qualify; per-chunk score matmuls cannot).
  - GPSIMD cannot access PSUM (BIR verifier), so the Pool engine cannot
    help with exp/copies (all loop data flows through PSUM).
  - With QK at full rate the wall is ACT exp + cross-engine slot-recycle
    latency; exp is split ACT 10 chunks / DVE-Schraudolph 6 chunks
    (engine "D": one fused tensor_scalar i16 = trunc(A*s + B) bit-viewed
    as fp16, bias calibrated unbiased; ~1.1e-2 total rel err), with
    engines colored around the 3-slot recycle cycle so a slot's previous
    reader is never the same engine as its next producer.
"""

import math
import os

os.environ.setdefault("MYCRO_LOCAL_CACHE", "1")

import numpy as np

import concourse.bacc as bacc
import concourse.bass as bass
import concourse.mybir as mybir
import concourse.tile as tile
from concourse.bass_utils import run_bass_kernel_spmd

# Problem constants (hardcoded per contract)
B, H, S, D = 2, 12, 2048, 64
M = 300                      # memory expansion length
NCORES = 8
HPC = (B * H) // NCORES      # 3 heads per core
SCALE = 1.0 / math.sqrt(768.0)

NCH = S // 128               # 16 text key chunks
QB = 512                     # queries per block
NQB = S // QB                # 4 query blocks

F32 = mybir.dt.float32
F16 = mybir.dt.float16
F8 = mybir.dt.float8e4
I16 = mybir.dt.int16
EXP = mybir.ActivationFunctionType.Exp
DR = mybir.MatmulPerfMode.DoubleRow

# Schraudolph exp: i16 = trunc(s*A + B) viewed as fp16. A maps raw scores
# through the softmax scale into fp16-exponent units; B centers the
# mantissa-linearization sawtooth so schrau chunks are unbiased vs ACT
# exp chunks (calibrated numerically over the score distribution).
A_SCH = (1024.0 / math.log(2.0)) * SCALE
B_SCH = 15360.0 - 57.25

# Engine schedule over the 16 chunks: A = ACT exp, D = DVE schraudolph.
# 6 bracketed groups (3,3,2,3,3,2) rotating 2 3-bank PSUM slots. QK
# matmuls within a group share one start/stop accumulation bracket
# (disjoint regions of the slot tile) -- on HW the PE is SW-decode bound
# (~71ns/instruction) and independent start/stop matmuls cost ~2x their
# compute, while bracketed/chained ones run at compute speed. The PE
# stream is kept pure-fp16: mixing DoubleRow-fp8 and fp16 matmuls costs
# ~1us per mode switch on HW (measured), which is why fp8 QK lost.
_ENG = {2: "D", 5: "D", 7: "D", 10: "D", 13: "D", 15: "D"}
GROUPS = [(_ENG.get(i, "A"), 1) for i in range(16)]
assert sum(n for _, n in GROUPS) == NCH


def build_program(loop_n=None):
    nc = bacc.Bacc("TRN2", target_bir_lowering=False, debug=False)

    qT_d = nc.dram_tensor("qT", [HPC, D, S], F16, kind="ExternalInput")
    kT_d = nc.dram_tensor("kT", [HPC, D, S], F16, kind="ExternalInput")
    vT_d = nc.dram_tensor("vT", [HPC, D, S], F16, kind="ExternalInput")
    wq_d = nc.dram_tensor("Wq", [D, D], F16, kind="ExternalInput")
    wk_d = nc.dram_tensor("Wk", [D, D], F16, kind="ExternalInput")
    wv_d = nc.dram_tensor("Wv", [D, D], F16, kind="ExternalInput")
    # memory tokens pre-chunked on host: [128, 3, 65], zero-padded rows.
    # mkaug = [mk | 1], maug = [mv | 1] (ones only on the 300 valid rows).
    mkaug_d = nc.dram_tensor("mkaug", [128, 3, D + 1], F16, kind="ExternalInput")
    maug_d = nc.dram_tensor("maug", [128, 3, D + 1], F16, kind="ExternalInput")
    id_d = nc.dram_tensor("ident", [128, 128], F32, kind="ExternalInput")
    out_d = nc.dram_tensor("out", [HPC, S, D], F32, kind="ExternalOutput")

    with tile.TileContext(nc) as tc:
        with (
            tc.tile_pool(name="const", bufs=1) as constp,
            tc.tile_pool(name="raw", bufs=HPC) as rawp,
            tc.tile_pool(name="proj", bufs=HPC) as projp,
            tc.tile_pool(name="ptp", bufs=6) as ptp,
            tc.tile_pool(name="sm", bufs=3) as smp,
            tc.tile_pool(name="psS", bufs=6, space="PSUM") as psS,
            tc.tile_pool(name="psO", bufs=2, space="PSUM") as psO,
        ):
            # ---- constants (small, issued first on the DMA queue) ----
            wq_s = constp.tile([D, D], F16, tag="wq")
            nc.sync.dma_start(out=wq_s, in_=wq_d[:])
            wk_s = constp.tile([D, D], F16, tag="wk")
            nc.sync.dma_start(out=wk_s, in_=wk_d[:])
            wv_s = constp.tile([D, D], F16, tag="wv")
            nc.sync.dma_start(out=wv_s, in_=wv_d[:])
            id_s = constp.tile([128, 128], F32, tag="id")
            nc.sync.dma_start(out=id_s, in_=id_d[:])
            mkaug_s = constp.tile([128, 3, D + 1], F16, tag="mkaug")
            nc.sync.dma_start(out=mkaug_s, in_=mkaug_d[:])
            maug_s = constp.tile([128, 3, D + 1], F16, tag="maug")
            nc.sync.dma_start(out=maug_s, in_=maug_d[:])

            # preload the exp table set early (overlaps initial DMA)
            warm = smp.tile([1, 1], F32, tag="warm", bufs=1)
            nc.vector.memset(warm, 0.0)
            nc.scalar.activation(warm, warm, EXP)

            # ---- M2 = [scale*mk ; 1]^T [mv | 1]  (65x65, shared) ----
            # raw = mkaug^T maug accumulated over the 3 memory chunks;
            # rows 0-63 then get *SCALE, row 64 (the 1^T part) copied as-is.
            m2_ps = psS.tile([D + 1, D + 1], F32, tag="sc", name="m2ps")
            for j in range(3):
                nc.tensor.matmul(m2_ps, mkaug_s[:, j], maug_s[:, j],
                                 start=(j == 0), stop=(j == 2))
            M2 = constp.tile([128, D + 1], F16, tag="m2")
            nc.vector.memset(M2, 0.0)
            nc.vector.tensor_scalar_mul(M2[0:D], m2_ps[0:D], SCALE)
            nc.vector.tensor_copy(out=M2[D:D + 1], in_=m2_ps[D:D + 1])
            idh = constp.tile([D + 1, D + 1], F16, tag="idh")
            nc.vector.tensor_copy(out=idh, in_=id_s[0:D + 1, 0:D + 1])

            QTb = [None] * HPC   # [128, S] fp16: Q^T, ones row 64, 0 pad
            QT8 = [None] * HPC   # [128, 2, S] fp8: planes (Q8, Q8), 0 pad
            KT8 = [None] * HPC   # [128, 2, S] fp8: planes (K8, K-K8), 0 pad
            V = [None] * HPC     # [128, 16, 65] fp16: V chunks, ones col
            raws = [None] * HPC

            def load_head(h):
                qT_s = rawp.tile([D, S], F16, tag="qraw", name=f"qraw{h}")
                nc.sync.dma_start(out=qT_s, in_=qT_d[h])
                kT_s = rawp.tile([D, S], F16, tag="kraw", name=f"kraw{h}")
                nc.sync.dma_start(out=kT_s, in_=kT_d[h])
                vT_s = rawp.tile([D, S], F16, tag="vraw", name=f"vraw{h}")
                nc.sync.dma_start(out=vT_s, in_=vT_d[h])
                raws[h] = (qT_s, kT_s, vT_s)
                QTb[h] = projp.tile([128, S], F16, tag="QTb", name=f"QTb{h}")
                QT8[h] = projp.tile([128, 2, S], F8, tag="QT8", name=f"QT8{h}")
                KT8[h] = projp.tile([128, 2, S], F8, tag="KT8", name=f"KT8{h}")
                nc.vector.memset(QTb[h], 0.0)
                nc.vector.memset(QT8[h], 0.0)
                nc.vector.memset(KT8[h], 0.0)
                V[h] = projp.tile([128, NCH, D + 1], F16, tag="V", name=f"V{h}")
                nc.vector.memset(QTb[h][D:D + 1], 1.0)
                nc.vector.memset(V[h][:, :, D:D + 1], 1.0)

            def proj_subtasks(h):
                """12 PSUM-group subtasks projecting head h; one per pipeline
                item so pool-slot rotations never stall the score pipeline."""
                qT_s, kT_s, vT_s = raws[h]

                def mk_q(i):
                    def run():
                        sl = slice(i * QB, (i + 1) * QB)
                        ps = psS.tile([128, QB], F32, tag="sc",
                                      name=f"pjq{h}_{i}")
                        nc.tensor.matmul(ps[0:D], wq_s, qT_s[:, sl],
                                         start=True, stop=True)
                        nc.vector.tensor_copy(out=QTb[h][0:D, sl], in_=ps[0:D])
                        nc.vector.tensor_copy(out=QT8[h][0:D, 0, sl],
                                              in_=ps[0:D])
                        nc.vector.tensor_copy(out=QT8[h][0:D, 1, sl],
                                              in_=ps[0:D])
                    return run

                def mk_k(i):
                    def run():
                        sl = slice(i * QB, (i + 1) * QB)
                        ps = psS.tile([128, QB], F32, tag="sc",
                                      name=f"pjk{h}_{i}")
                        nc.tensor.matmul(ps[0:D], wk_s, kT_s[:, sl],
                                         start=True, stop=True)
                        nc.vector.tensor_copy(out=KT8[h][0:D, 0, sl],
                                              in_=ps[0:D])
                        # plane 1 = requantized fp8 residual (K - fp8(K))
                        nc.vector.tensor_tensor(
                            out=KT8[h][0:D, 1, sl], in0=ps[0:D],
                            in1=KT8[h][0:D, 0, sl],
                            op=mybir.AluOpType.subtract)
                    return run

                def mk_v(g):
                    def run():
                        ps_v = psS.tile([128, 4 * D], F32, tag="sc",
                                        name=f"pjv{h}_{g}")
                        for j in range(4):
                            i = 4 * g + j
                            nc.tensor.matmul(
                                ps_v[:, j * D:(j + 1) * D],
                                vT_s[:, i * 128:(i + 1) * 128], wv_s,
                                start=(j == 0), stop=(j == 3))
                        nc.vector.tensor_copy(
                            out=V[h][:, 4 * g:4 * g + 4, 0:D],
                            in_=ps_v.rearrange("p (a b) -> p a b", a=4))
                    return run

                ts = []
                for i in range(NQB):
                    ts.append(mk_q(i))
                    ts.append(mk_k(i))
                for g in range(4):
                    ts.append(mk_v(g))
                return ts

            # chunk base index per group
            GBASE = []
            c0 = 0
            for _, n in GROUPS:
                GBASE.append(c0)
                c0 += n

            # ---- flat attention pipeline over (h, qb, group) ----
            items = [(h, qb, gi) for h in range(HPC) for qb in range(NQB)
                     for gi in range(len(GROUPS))]

            state = {}  # per (h,qb): dict(outT=...)

            def emit_qk_exp(h, qb, gi):
                eng, glen = GROUPS[gi]
                qsl = slice(qb * QB, (qb + 1) * QB)
                sc = psS.tile([128, glen, QB], F32, tag="sc",
                              name=f"sc{h}_{qb}_{gi}")
                for ci in range(glen):
                    c = GBASE[gi] + ci
                    nc.tensor.matmul(
                        sc[:, ci, :],
                        KT8[h][:, :, c * 128:(c + 1) * 128],
                        QT8[h][:, :, qsl],
                        start=True, stop=True, perf_mode=DR)
                pt = ptp.tile([128, glen, QB], F16, tag="pt",
                              name=f"pt{h}_{qb}_{gi}")
                if eng == "A":
                    nc.scalar.activation(pt, sc, EXP, scale=SCALE)
                else:
                    nc.vector.tensor_scalar(
                        pt.bitcast(I16), sc, A_SCH, B_SCH,
                        mybir.AluOpType.mult, mybir.AluOpType.add)
                return pt

            def emit_pv(h, qb, gi, pt):
                glen = GROUPS[gi][1]
                st = state[(h, qb)]
                if st["outT"] is None:
                    st["outT"] = psO.tile([D + 1, QB], F32, tag="o",
                                          name=f"o{h}_{qb}")
                for ci in range(glen):
                    c = GBASE[gi] + ci
                    nc.tensor.matmul(
                        st["outT"], V[h][:, c, :], pt[:, ci, :],
                        start=(c == 0), stop=False)

            def emit_m2_copy(h, qb):
                """Close the outT accumulation with the memory-token term,
                then stage outT to SBUF (DVE) for the PE transposes."""
                qsl = slice(qb * QB, (qb + 1) * QB)
                outT = state[(h, qb)]["outT"]
                nc.tensor.matmul(outT, M2, QTb[h][:, qsl],
                                 start=False, stop=True)
                outT_sb = smp.tile([D + 1, QB], F16, tag="osb",
                                   name=f"osb{h}_{qb}")
                nc.vector.tensor_copy(out=outT_sb, in_=outT)
                return outT_sb

            def emit_norm(h, qb, outT_sb):
                tr = psS.tile([128, NQB, D + 4], F16, tag="sc",
                              name=f"tr{h}_{qb}")
                rec = smp.tile([128, NQB, 1], F32, tag="rec",
                               name=f"rec{h}_{qb}")
                of = smp.tile([128, NQB, D], F32, tag="of",
                              name=f"of{h}_{qb}")
                for j in range(QB // 128):
                    nc.tensor.matmul(
                        tr[:, j, 0:D + 1], outT_sb[:, j * 128:(j + 1) * 128],
                        idh, is_transpose=True,
                        start=(j == 0), stop=(j == QB // 128 - 1))
                for j in range(QB // 128):
                    nc.vector.reciprocal(rec[:, j], tr[:, j, D:D + 1])
                    nc.vector.tensor_scalar_mul(of[:, j], tr[:, j, 0:D],
                                                rec[:, j])
                    r0 = qb * QB + j * 128
                    nc.sync.dma_start(out=out_d[h, r0:r0 + 128, :],
                                      in_=of[:, j])

            def drive(todo):
                """Flat software pipeline. Per item: QK+exp for group i, PV
                for group i-2 (two-group lag so the batched exp instruction
                never blocks PE), plus the deferred normalize chain."""
                state.clear()
                pipe = []        # queue of (h, qb, gi, pt) awaiting PV
                pend = None      # (h, qb, outT_sb) awaiting transposes
                LAG = 4

                def retire(entry):
                    nonlocal pend
                    ph, pqb, pgi, ppt = entry
                    emit_pv(ph, pqb, pgi, ppt)
                    if pend is not None:
                        emit_norm(*pend)
                        pend = None
                    if pgi == len(GROUPS) - 1:
                        pend = (ph, pqb, emit_m2_copy(ph, pqb))

                for gidx, (h, qb, gi) in enumerate(items):
                    if (h, qb) not in state:
                        state[(h, qb)] = {"outT": None}
                    pt = emit_qk_exp(h, qb, gi)
                    pipe.append((h, qb, gi, pt))
                    if len(pipe) > LAG:
                        retire(pipe.pop(0))
                    # drip one projection subtask per item, starting mid-qb0
                    # so the h1 raw DMAs land before PE reaches these matmuls
                    if gidx >= 3 and todo:
                        todo.pop(0)()
                while pipe:
                    retire(pipe.pop(0))
                if pend is not None:
                    emit_norm(*pend)

            if loop_n is None:
                # graded path: h0 projects upfront; h1/h2 projections are
                # drip-fed into the pipeline while their DMAs stream in
                load_head(0)
                for t in proj_subtasks(0):
                    t()
                load_head(1)
                load_head(2)
                drive(proj_subtasks(1) + proj_subtasks(2))
            else:
                # timing path: everything projected upfront, then the whole
                # attention pipeline repeats loop_n times in a HW loop.
                for h in range(HPC):
                    load_head(h)
                for h in range(HPC):
                    for t in proj_subtasks(h):
                        t()
                with tc.For_i(0, loop_n, 1, hint_engines=(
                        mybir.EngineType.PE, mybir.EngineType.Activation,
                        mybir.EngineType.DVE)):
                    drive([])

    nc.compile()
    return nc


_PROG = None


def _get_prog():
    global _PROG
    if _PROG is None:
        _PROG = build_program()
    return _PROG


def make_in_maps(q, k, v, Wq, bq, Wk, bk, Wv, bv, memory_k, memory_v):
    for b_ in (bq, bk, bv):
        assert np.allclose(np.asarray(b_), 0.0), "nonzero bias not supported"
    f32 = np.float32
    f16 = np.float16
    qh = np.asarray(q, f32).reshape(B * H, S, D)
    kh = np.asarray(k, f32).reshape(B * H, S, D)
    vh = np.asarray(v, f32).reshape(B * H, S, D)
    mk = np.asarray(memory_k, f32)[0, 0].astype(f16)   # [300, 64]
    mv = np.asarray(memory_v, f32)[0, 0].astype(f16)
    mkaug = np.zeros((128, 3, D + 1), f16)
    maug = np.zeros((128, 3, D + 1), f16)
    for j in range(3):
        rows = min(128, M - j * 128)
        mkaug[0:rows, j, 0:D] = mk[j * 128:j * 128 + rows]
        mkaug[0:rows, j, D] = 1.0
        maug[0:rows, j, 0:D] = mv[j * 128:j * 128 + rows]
        maug[0:rows, j, D] = 1.0
    shared = {
        "Wq": np.ascontiguousarray(np.asarray(Wq, f16)),
        "Wk": np.ascontiguousarray(np.asarray(Wk, f16)),
        "Wv": np.ascontiguousarray(np.asarray(Wv, f16)),
        "mkaug": mkaug,
        "maug": maug,
        "ident": np.eye(128, dtype=f32),
    }
    in_maps = []
    for c in range(NCORES):
        sl = slice(c * HPC, (c + 1) * HPC)
        in_maps.append({
            "qT": np.ascontiguousarray(qh[sl].transpose(0, 2, 1).astype(f16)),
            "kT": np.ascontiguousarray(kh[sl].transpose(0, 2, 1).astype(f16)),
            "vT": np.ascontiguousarray(vh[sl].transpose(0, 2, 1).astype(f16)),
            **shared,
        })
    return in_maps


def _assemble(results):
    outs = [results[c]["out"] for c in range(NCORES)]
    return np.concatenate(outs, axis=0).reshape(B, H, S, D)


_EXEC = None  # cached jitted executable: repeat kernel() calls skip re-trace


def _get_exec():
    """Build the sharded PJRT executable once (mirrors bass2jax's axon path
    in run_bass_kernel_spmd, but keeps the jitted callable so repeated
    kernel() invocations pay only input upload + execution)."""
    global _EXEC
    if _EXEC is not None:
        return _EXEC
    import jax
    from jax.experimental.shard_map import shard_map
    from jax.sharding import Mesh, PartitionSpec
    from concourse import bass2jax

    nc = _get_prog()
    bass2jax.install_neuronx_cc_hook()
    partition_name = (nc.partition_id_tensor.name
                      if nc.partition_id_tensor else None)
    in_names, out_names, out_avals, zero_shapes = [], [], [], []
    for alloc in nc.m.functions[0].allocations:
        if not isinstance(alloc, mybir.MemoryLocationSet):
            continue
        name = alloc.memorylocations[0].name
        if alloc.kind == "ExternalInput":
            if name != partition_name:
                in_names.append(name)
        elif alloc.kind == "ExternalOutput":
            out_names.append(name)
            shape = tuple(alloc.tensor_shape)
            dtype = mybir.dt.np(alloc.dtype)
            out_avals.append(jax.core.ShapedArray(shape, dtype))
            zero_shapes.append((shape, dtype))
    n_params = len(in_names)
    all_in_names = list(in_names) + list(out_names)
    if partition_name is not None:
        all_in_names.append(partition_name)

    def _body(*args):
        operands = list(args)
        if partition_name is not None:
            operands.append(bass2jax.partition_id_tensor())
        return tuple(bass2jax._bass_exec_p.bind(
            *operands,
            out_avals=tuple(out_avals),
            in_names=tuple(all_in_names),
            out_names=tuple(out_names),
            lowering_input_output_aliases=(),
            sim_require_finite=True,
            sim_require_nnan=True,
            nc=nc,
        ))

    devices = jax.devices()[:NCORES]
    mesh = Mesh(np.asarray(devices), ("core",))
    n_outs = len(out_names)
    in_specs = (PartitionSpec("core"),) * (n_params + n_outs)
    out_specs = (PartitionSpec("core"),) * n_outs
    sharded = jax.jit(
        shard_map(_body, mesh=mesh, in_specs=in_specs, out_specs=out_specs,
                  check_rep=False),
        donate_argnums=tuple(range(n_params, n_params + n_outs)),
        keep_unused=True)
    _EXEC = (sharded, in_names, out_names, out_avals, zero_shapes)
    return _EXEC


def kernel(**inputs):
    sharded, in_names, out_names, out_avals, zero_shapes = _get_exec()
    in_maps = make_in_maps(**inputs)
    concat_in = [
        np.concatenate([in_maps[c][name] for c in range(NCORES)], axis=0)
        for name in in_names
    ]
    zeros = [np.zeros((NCORES * s[0], *s[1:]), d) for s, d in zero_shapes]
    out_arrs = sharded(*concat_in, *zeros)
    results = [
        {name: np.asarray(out_arrs[i]).reshape(
            NCORES, *out_avals[i].shape)[c]
         for i, name in enumerate(out_names)}
        for c in range(NCORES)
    ]
    return _assemble(results)


def kernel_timed(**inputs):
    """Returns (output, exec_time_ns or None). Used by test.py."""
    nc = _get_prog()
    in_maps = make_in_maps(**inputs)
    try:
        res = run_bass_kernel_spmd(nc, in_maps, list(range(NCORES)), trace=True)
        return _assemble(res.results), res.exec_time_ns
    except ModuleNotFoundError:
        res = run_bass_kernel_spmd(nc, in_maps, list(range(NCORES)))
        return _assemble(res.results), None
